# revision 1
# baseline (speedup 1.0000x reference)
"""GAT (3-layer, PyG-style) Trainium2 Bass kernel, 8-core dst-sharded.

Self-contained: takes full inputs, shards internally, returns full output.

Design:
  - dst nodes sharded across 8 cores (graph parallel per the sharding hint).
  - Per layer (3 SPMD launches; host only shards/permutes/transposes/casts
    between them):
    dense phase: node-major DRAM gather-table rows [hW*bn_s (bf16) | al_s f32]
      built by PE matmuls (bf16 lhsT = h^T chunks, rhs = W_aug = [W | W@a_s |
      W@a_d]), al_s carried in-row for ALL layers (row = 128 bf16 for the
      64-wide layers, 384 bf16 for the concat=False layer), plus dense al_d.
    edge phase: padded-CSR slots (K slots per dst per src-half, K bucketed by
      max src-half degree), dma_gather of one table row per slot,
      ex = exp(leaky_relu(al_s + al_d)) written back into the row, messages
      scaled by ex, segment-sum via per-k wide selection matrices (column-
      sliced per shift) matmul-accumulating [msg_sums | sum_ex] per 128-dst
      window in PSUM; windows drained in batches (7/bank or 3 banks) to
      amortize DVE overheads; drain divides by sum_ex and adds skip matmul +
      bias (+BN fold, +ReLU on the scalar engine).
  - src space is split into lo/hi halves with separate table tensors so
    dma_gather's int16 indices stay < 32768 and edge gathers can overlap the
    tail of the dense phase; padded slots gather a sentinel row whose
    al_s = -40 (=> ex ~ 0) and whose message contribution ~ 0.
"""
import numpy as np
import ml_dtypes

import concourse.bacc as bacc
import concourse.mybir as mybir
import concourse.tile as tile
from concourse.alu_op_type import AluOpType
from concourse.bass_utils import run_bass_kernel_spmd

BF16 = mybir.dt.bfloat16
F32 = mybir.dt.float32
I16 = mybir.dt.int16

NC = 8
KLIST = (8, 16, 32, 64, 128)
P = 128
EPS = 1e-5
SENT_ALS = -40.0


def _round_up(x, m):
    return (x + m - 1) // m * m


# ----------------------------------------------------------------- planning

def build_plan(src, dst, N):
    D = N // NC
    HALF = N // 2
    core = dst // D
    dloc = dst % D
    half = (src >= HALF).astype(np.int64)

    deg = np.zeros((NC, D, 2), np.int64)
    np.add.at(deg, (core, dloc, half), 1)
    mx = deg.max(axis=2)  # [NC, D]
    Kd = np.select([mx <= 8, mx <= 16, mx <= 32, mx <= 64], [8, 16, 32, 64], 128)
    assert mx.max() <= 128, f"degree bucket overflow: {mx.max()}"

    nK = {k: _round_up(int((Kd == k).sum(axis=1).max()), 16) for k in KLIST}
    Dp = sum(nK.values())
    nK[8] += (-Dp) % 128
    Dp = sum(nK.values())
    off = {}
    o = 0
    for k in KLIST:
        off[k] = o
        o += nK[k]

    slabs = []
    for k in KLIST:
        q = P // k
        for i in range(nK[k] * k // P):
            slabs.append((k, off[k] + i * q))
    nslab = len(slabs)

    nwin = Dp // P
    slot0 = np.zeros(Dp, np.int64)
    for si, (k, vd0) in enumerate(slabs):
        q = P // k
        for j in range(q):
            slot0[vd0 + j] = si * P + j * k
    TOT = _round_up(nslab, 64) * P

    shared = dict(N=N, D=D, HALF=HALF, Dp=Dp, slabs=slabs,
                  nwin=nwin, TOT=TOT)

    plans = []
    for c in range(NC):
        vid = np.full(D, -1, np.int64)
        vmap = np.full(Dp, -1, np.int64)
        used = {k: 0 for k in KLIST}
        order = np.argsort(Kd[c], kind="stable")
        for d in order:
            k = int(Kd[c, d])
            pos = off[k] + used[k]
            used[k] += 1
            vid[d] = pos
            vmap[pos] = d
        em = core == c
        es = src[em]
        evd = vid[dloc[em]]
        eh = half[em]
        key = evd * 2 + eh
        si = np.argsort(key, kind="stable")
        ks = key[si]
        starts = np.zeros(2 * Dp + 1, np.int64)
        np.cumsum(np.bincount(ks, minlength=2 * Dp), out=starts[1:])
        rank = np.arange(len(ks)) - starts[ks]
        spos = slot0[evd[si]] + rank
        essorted = es[si]
        lo = np.full(TOT, HALF, np.int64)
        hi = np.full(TOT, HALF, np.int64)
        mlo = ks % 2 == 0
        lo[spos[mlo]] = essorted[mlo]
        hi[spos[~mlo]] = essorted[~mlo] - HALF
        plans.append(dict(vmap=vmap, idx_lo=_wrap16(lo), idx_hi=_wrap16(hi)))
    return shared, plans


def _wrap16(stream):
    TOT = len(stream)
    w = stream.reshape(TOT // 16, 16).T.astype(np.int16)
    return np.tile(w, (8, 1))


def _svar_wide():
    """One wide [P, 2P] selection matrix per k; the (k, s)-shifted selection
    matrix is the column window [P - s*q, 2P - s*q) of wide_k, since
    wide_k[p, P + p//k] = 1 puts slot p's target at col s*q + p//k within
    that window."""
    mats = []
    for k in KLIST:
        m = np.zeros((P, 2 * P), np.float32)
        for p in range(P):
            m[p, P + p // k] = 1.0
        mats.append(m)
    return np.concatenate(mats, 1)  # [P, NKL*2P], partition-major contiguous


SVAR_NP = _svar_wide()
NKL = len(KLIST)


# ------------------------------------------------------------- kernel build

def build_layer(shared, F, OUTW, relu, mean_heads, tcap, gb, dbg_stage=99):
    N, HALF, Dp = shared["N"], shared["HALF"], shared["Dp"]
    nwin = shared["nwin"]
    slabs = shared["slabs"]
    TOT = shared["TOT"]
    TOT_lo = TOT_hi = TOT
    nslab = len(slabs)
    first_slab = {}
    last_slab = {}
    for i, (k, vd0) in enumerate(slabs):
        w = vd0 // P
        first_slab.setdefault(w, i)
        last_slab[w] = i
    groups = []
    s0 = 0
    while s0 < nslab:
        groups.append((s0, min(s0 + tcap, nslab)))
        s0 += tcap

    RW = 128 if OUTW == 64 else 384
    NA = 72 if OUTW == 64 else 264
    NAW = OUTW + 4
    PADW = 128 if OUTW == 64 else 512
    Npad = _round_up(N, P)
    nchunk = Npad // P
    TROWS = Npad + 2

    nc = bacc.Bacc("TRN2", target_bir_lowering=False, debug=False)
    hT = nc.dram_tensor("hT", [F, Npad], BF16, kind="ExternalInput")
    hTow = nc.dram_tensor("hTow", [F, Dp], BF16, kind="ExternalInput")
    Waug = nc.dram_tensor("Waug", [F, NA], BF16, kind="ExternalInput")
    skipW = nc.dram_tensor("skipW", [F, 64], BF16, kind="ExternalInput")
    biasR = nc.dram_tensor("biasR", [P, 64], F32, kind="ExternalInput")
    svar_in = nc.dram_tensor("svar", [P, NKL * 2 * P], BF16,
                             kind="ExternalInput")
    rep_in = nc.dram_tensor("rep", [len(KLIST) * 16, P], F32, kind="ExternalInput")
    sent_in = nc.dram_tensor("sent", [2, RW], BF16, kind="ExternalInput")
    idx_lo = nc.dram_tensor("idx_lo", [P, TOT_lo // 16], I16,
                            kind="ExternalInput")
    idx_hi = nc.dram_tensor("idx_hi", [P, TOT_hi // 16], I16,
                            kind="ExternalInput")

    TROWS_H = Npad - HALF + 2
    table_lo = nc.dram_tensor("table_lo", [TROWS_H, RW], BF16, kind="Internal")
    table_hi = nc.dram_tensor("table_hi", [TROWS_H, RW], BF16, kind="Internal")
    aldv_d = nc.dram_tensor("aldv", [Dp, 4], F32, kind="Internal")
    y_out = nc.dram_tensor("y", [P, nwin * 64], F32, kind="ExternalOutput")

    def table_row_ranges(n0, n1):
        """split [n0,n1) at HALF into (tensor, node range, dram row) pieces."""
        out = []
        cuts = sorted({n0, min(max(HALF, n0), n1), n1})
        for a, b in zip(cuts, cuts[1:]):
            if a >= b:
                continue
            if a < HALF:
                out.append((table_lo, a, b, a))
            else:
                out.append((table_hi, a, b, a - HALF))
        return out

    with tile.TileContext(nc) as tc:
        with (
            tc.tile_pool(name="const", bufs=1) as cp,
            tc.tile_pool(name="ybuf", bufs=1) as yp,
        ):
            waug_sb = cp.tile([F, NA], BF16)
            nc.sync.dma_start(waug_sb[:], Waug[:])
            skipw_sb = cp.tile([F, 64], BF16)
            nc.sync.dma_start(skipw_sb[:], skipW[:])
            bias_sb = cp.tile([P, 1, 64], F32)
            nc.sync.dma_start(bias_sb[:],
                              biasR[:].rearrange("p (x c) -> p x c", x=1))
            svar_sb = cp.tile([P, NKL * 2 * P], BF16)
            nc.sync.dma_start(svar_sb[:], svar_in[:])
            rep_sb = cp.tile([16, len(KLIST), P], F32)
            nc.sync.dma_start(rep_sb[:],
                              rep_in[:].rearrange("(v p) c -> p v c", p=16))
            hTow_sb = cp.tile([F, Dp], BF16)
            nc.scalar.dma_start(hTow_sb[:], hTow[:])
            y_sb = yp.tile([P, nwin, 64], F32)

            # ---------------- dense phase: gather table + dense al_d
            with (
                tc.tile_pool(name="dstage", bufs=3) as dsp,
                tc.tile_pool(name="pdense", bufs=2, space="PSUM") as pd,
                tc.tile_pool(name="pal", bufs=1, space="PSUM") as pal,
            ):
                sent_sb = dsp.tile([2, RW], BF16, tag="sent")
                nc.scalar.dma_start(sent_sb[:], sent_in[:])
                nc.scalar.dma_start(table_lo[HALF: HALF + 1, :],
                                    sent_sb[0:1, :])
                nc.scalar.dma_start(table_hi[HALF: HALF + 1, :],
                                    sent_sb[1:2, :])

                ndc = Dp // P
                alps = pal.tile([P, ndc * 4], F32, space="PSUM")
                for i in range(ndc):
                    nc.tensor.matmul(
                        alps[:, i * 4: (i + 1) * 4],
                        hTow_sb[:, i * P: (i + 1) * P],
                        waug_sb[:, NA - 4: NA],
                        start=True, stop=True,
                    )
                alsb = dsp.tile([P, ndc * 4], F32, tag="alsb")
                nc.vector.tensor_copy(alsb[:], alps[:])
                nc.scalar.dma_start(
                    aldv_d[:].rearrange("(i p) h -> p i h", p=P),
                    alsb[:].rearrange("p (i h) -> p i h", h=4),
                )

                UW = OUTW + 8  # used row prefix: msg bf16 + 4 f32 al_s
                sgb = 4 * gb
                for sg0 in range(0, nchunk, sgb):
                    sg1 = min(sg0 + sgb, nchunk)
                    stage = dsp.tile([F, sgb * P], BF16, tag="stage")
                    nc.sync.dma_start(stage[:, : (sg1 - sg0) * P],
                                      hT[:, sg0 * P: sg1 * P])
                    tstage = dsp.tile([P, sgb, RW], BF16, tag="tstage")
                    tf32 = tstage[:].bitcast(F32)
                    for g0 in range(sg0, sg1, gb):
                        g1 = min(g0 + gb, sg1)
                        ng = g1 - g0
                        c0 = g0 - sg0
                        dps = pd.tile([P, gb * PADW], F32, space="PSUM",
                                      tag="dps")
                        for i in range(ng):
                            nc.tensor.matmul(
                                dps[:, i * PADW: i * PADW + NA],
                                stage[:, (c0 + i) * P: (c0 + i + 1) * P],
                                waug_sb[:],
                                start=True, stop=True,
                            )
                        dv = dps[:].rearrange("p (i w) -> p i w", w=PADW)
                        if mean_heads:
                            # layer 2's dense phase is DVE-copy-bound; the
                            # Activation engine is idle here
                            nc.scalar.activation(
                                tstage[:, c0: c0 + ng, 0:OUTW],
                                dv[:, :ng, 0:OUTW],
                                mybir.ActivationFunctionType.Copy)
                        else:
                            nc.vector.tensor_copy(
                                tstage[:, c0: c0 + ng, 0:OUTW],
                                dv[:, :ng, 0:OUTW])
                        nc.vector.tensor_copy(
                            tf32[:, c0: c0 + ng, OUTW // 2: OUTW // 2 + 4],
                            dv[:, :ng, OUTW: OUTW + 4])
                    for (tbl, a, b, r) in table_row_ranges(
                            sg0 * P, min(sg1 * P, N)):
                        # table_hi goes out on the Activation queue so the
                        # lo gathers' DMA-sem wait only covers table_lo
                        eng = (nc.scalar if tbl is table_hi
                               and not mean_heads else nc.sync)
                        # emit aligned middle as one DMA; partial chunks solo
                        n0 = a
                        while n0 < b:
                            if n0 % P != 0 or b - n0 < P:
                                n1 = min(b, n0 - n0 % P + P)
                                ci = n0 // P - sg0
                                eng.dma_start(
                                    tbl[r + n0 - a: r + n1 - a, 0:UW],
                                    tstage[n0 % P: n0 % P + (n1 - n0), ci,
                                           0:UW],
                                )
                            else:
                                n1 = n0 + (b - n0) // P * P
                                ci = n0 // P - sg0
                                m = (n1 - n0) // P
                                eng.dma_start(
                                    tbl[r + n0 - a: r + n1 - a, 0:UW]
                                    .rearrange("(i p) w -> p i w", p=P),
                                    tstage[:, ci: ci + m, 0:UW],
                                )
                            n0 = n1

            # ---------------- edge phase
            NB = 3 if mean_heads else 7          # windows per drain batch
            BSTRIDE = 512 if mean_heads else 68  # f32 cols per window slot
            with (
                tc.tile_pool(name="gpool",
                             bufs=2 if mean_heads else 3) as gp,
                tc.tile_pool(name="spool", bufs=2) as ssp,
                tc.tile_pool(name="pwin", bufs=2, space="PSUM") as pw,
                tc.tile_pool(name="palde", bufs=1 if mean_heads else 2,
                             space="PSUM") as pa,
                tc.tile_pool(name="psk", bufs=1 if mean_heads else 2,
                             space="PSUM") as pk,
            ):
                win_ps = {}
                for (s0, s1) in groups:
                    T = s1 - s0
                    g_lo = gp.tile([P, tcap, RW], BF16, tag="Glo")
                    g_hi = gp.tile([P, tcap, RW], BF16, tag="Ghi")
                    il_t = ssp.tile([P, tcap * 8], I16, tag="il")
                    ih_t = ssp.tile([P, tcap * 8], I16, tag="ih")
                    nc.scalar.dma_start(il_t[:, : T * 8],
                                        idx_lo[:, s0 * 8:(s0 + T) * 8])
                    nc.scalar.dma_start(ih_t[:, : T * 8],
                                        idx_hi[:, s0 * 8:(s0 + T) * 8])
                    nc.gpsimd.dma_gather(
                        g_lo[:, :T], table_lo[0: HALF + 1, :],
                        il_t[:, : T * 8], T * P, T * P, RW,
                        single_packet=False)
                    nc.gpsimd.dma_gather(
                        g_hi[:, :T], table_hi[0: HALF + 1, :],
                        ih_t[:, : T * 8], T * P, T * P, RW,
                        single_packet=False)

                    alde = ssp.tile([P, tcap, 4], F32, tag="alde")
                    i = s0
                    while i < s1:
                        k = slabs[i][0]
                        j = i
                        while j < s1 and slabs[j][0] == k:
                            j += 1
                        q = P // k
                        run = j - i
                        vb = slabs[i][1]
                        cont = ssp.tile([16, tcap, 4], F32, tag="cont")
                        nc.scalar.dma_start(
                            cont[:q, :run, :],
                            aldv_d[vb: vb + run * q, :].rearrange(
                                "(t j) h -> j t h", j=q),
                        )
                        aps = pa.tile([P, tcap * 4], F32, space="PSUM",
                                      tag="aldeps")
                        nc.tensor.matmul(
                            aps[:, : run * 4],
                            rep_sb[:q, KLIST.index(k), :],
                            cont[:q, :run, :].rearrange("j t h -> j (t h)"),
                            start=True, stop=True,
                        )
                        nc.vector.tensor_copy(
                            alde[:, i - s0: j - s0, :],
                            aps[:, : run * 4].rearrange("p (t h) -> p t h",
                                                        h=4),
                        )
                        i = j

                    z_t = ssp.tile([P, 2 * tcap, 4], F32, tag="z")
                    for h in range(2):
                        gs = (g_lo if h == 0 else g_hi)[:, :T, :]
                        gf = (g_lo if h == 0 else g_hi)[:].bitcast(F32)
                        zs = z_t[:, h * tcap: h * tcap + T, :]
                        nc.vector.tensor_tensor(
                            zs, gf[:, :T, OUTW // 2: OUTW // 2 + 4],
                            alde[:, :T, :], AluOpType.add)
                        nc.vector.scalar_tensor_tensor(
                            zs, zs, 0.2, zs, AluOpType.mult, AluOpType.max)
                        nc.scalar.activation(
                            gs[:, :, OUTW: OUTW + 4], zs,
                            mybir.ActivationFunctionType.Exp)
                        for hh in range(4):
                            ex_ap = gs[:, :, OUTW + hh: OUTW + hh + 1]
                            nc.vector.tensor_tensor(
                                gs[:, :, hh * (OUTW // 4):
                                   (hh + 1) * (OUTW // 4)],
                                gs[:, :, hh * (OUTW // 4):
                                   (hh + 1) * (OUTW // 4)],
                                ex_ap.to_broadcast([P, T, OUTW // 4]),
                                AluOpType.mult,
                            )

                    for i in range(s0, s1):
                        k, vd0 = slabs[i]
                        w = vd0 // P
                        b = w // NB
                        if b not in win_ps:
                            win_ps[b] = pw.tile([P, NB * BSTRIDE], F32,
                                                space="PSUM", tag="win",
                                                name=f"winb{b}")
                        wb = (w % NB) * BSTRIDE
                        off = P - (vd0 % P)
                        ki = KLIST.index(k)
                        sv = svar_sb[:, ki * 2 * P + off: ki * 2 * P + off + P]
                        for h in range(2):
                            st = (h == 0) and (first_slab[w] == i)
                            fin = (h == 1) and (last_slab[w] == i)
                            gh = g_lo if h == 0 else g_hi
                            t = i - s0
                            nc.tensor.matmul(
                                win_ps[b][:, wb: wb + NAW], sv,
                                gh[:, t, 0:NAW],
                                start=st, stop=fin, skip_group_check=True)
                        w1 = min((b + 1) * NB, nwin) - 1
                        if w != w1 or last_slab[w] != i or dbg_stage < 6:
                            continue
                        pwb = win_ps.pop(b)
                        w0 = b * NB
                        nb = w1 - w0 + 1
                        pv = pwb[:, : nb * BSTRIDE].rearrange(
                            "p (b c) -> p b c", c=BSTRIDE)
                        sk = pk.tile([P, NB * 64], F32, space="PSUM",
                                     tag="skps")
                        for j in range(nb):
                            nc.tensor.matmul(
                                sk[:, j * 64: (j + 1) * 64],
                                hTow_sb[:, (w0 + j) * P: (w0 + j + 1) * P],
                                skipw_sb[:], start=True, stop=True)
                        skv = sk[:, : nb * 64].rearrange(
                            "p (b c) -> p b c", c=64)
                        rec = ssp.tile([P, NB, 4], F32, tag="rec")
                        nc.vector.reciprocal(rec[:, :nb, :],
                                             pv[:, :, OUTW: OUTW + 4])
                        yw = y_sb[:, w0: w0 + nb, :]
                        if mean_heads:
                            m_t = ssp.tile([P, NB, 4, 64], F32, tag="mt")
                            for hh in range(4):
                                nc.vector.tensor_tensor(
                                    m_t[:, :nb, hh, :],
                                    pv[:, :, hh * 64: (hh + 1) * 64],
                                    rec[:, :nb, hh: hh + 1].to_broadcast(
                                        [P, nb, 64]),
                                    AluOpType.mult)
                            nc.vector.tensor_tensor(yw, m_t[:, :nb, 0, :],
                                                    m_t[:, :nb, 1, :],
                                                    AluOpType.add)
                            nc.vector.tensor_tensor(yw, yw, m_t[:, :nb, 2, :],
                                                    AluOpType.add)
                            nc.vector.tensor_tensor(yw, yw, m_t[:, :nb, 3, :],
                                                    AluOpType.add)
                            nc.vector.tensor_scalar_mul(yw, yw, 0.25)
                            nc.vector.tensor_tensor(yw, yw, skv, AluOpType.add)
                            nc.vector.tensor_tensor(
                                yw, yw, bias_sb[:].to_broadcast([P, nb, 64]),
                                AluOpType.add)
                        else:
                            for hh in range(4):
                                nc.vector.tensor_tensor(
                                    yw[:, :, hh * 16: (hh + 1) * 16],
                                    pv[:, :, hh * 16: (hh + 1) * 16],
                                    rec[:, :nb, hh: hh + 1].to_broadcast(
                                        [P, nb, 16]),
                                    AluOpType.mult)
                            nc.vector.tensor_tensor(yw, yw, skv, AluOpType.add)
                            nc.vector.tensor_tensor(
                                yw, yw, bias_sb[:].to_broadcast([P, nb, 64]),
                                AluOpType.add)
                            if relu:
                                nc.scalar.activation(
                                    yw, yw,
                                    mybir.ActivationFunctionType.Relu)
                        # write this batch's y slice now so the final DMA
                        # isn't a serial tail after the last drain
                        nc.sync.dma_start(
                            y_out[:, w0 * 64: (w0 + nb) * 64].rearrange(
                                "p (w c) -> p w c", c=64),
                            y_sb[:, w0: w0 + nb, :])
    nc.compile()
    return nc


# ------------------------------------------------------------------ driver

_CACHE = {}
_DBG = []
_EXEC_NS = []


def _blockdiag(a):
    H, C = a.shape
    m = np.zeros((H * C, H), np.float32)
    for hh in range(H):
        m[hh * C: (hh + 1) * C, hh] = a[hh]
    return m


def _sent01():
    row = np.zeros((2, 64), np.float32)
    row[:, 32:36] = SENT_ALS
    return row.view(np.uint16).view(ml_dtypes.bfloat16)  # [2, 128]


def kernel(**inp):
    x = np.asarray(inp["x"], np.float32)
    ei = np.asarray(inp["edge_index"], np.int64)
    N, IN = x.shape
    E = ei.shape[1]

    loops = np.arange(N, dtype=np.int64)
    src = np.concatenate([ei[0], loops])
    dst = np.concatenate([ei[1], loops])

    pkey = ("plan", N, E, hash(ei.tobytes()))
    if pkey not in _CACHE:
        _CACHE[pkey] = build_plan(src, dst, N)
    shared, plans = _CACHE[pkey]
    Dp, D = shared["Dp"], shared["D"]
    Npad = _round_up(N, P)

    def prep01(Wv, a_s, a_d, cb, sW, sb, g, b, m, v):
        Wv, sW = np.asarray(Wv, np.float32), np.asarray(sW, np.float32)
        bns = (np.asarray(g) / np.sqrt(np.asarray(v) + EPS)).astype(np.float32)
        bnt = (np.asarray(b) - np.asarray(m) * bns).astype(np.float32)
        Waug = np.concatenate(
            [Wv * bns[None, :], Wv @ _blockdiag(np.asarray(a_s)),
             Wv @ _blockdiag(np.asarray(a_d))], 1)
        return (Waug, sW * bns[None, :], np.asarray(cb) * bns
                + np.asarray(sb) * bns + bnt, _sent01(), None)

    def prep2(Wv, a_s, a_d, cb, sW, sb):
        Wv = np.asarray(Wv, np.float32)
        Waug = np.concatenate(
            [Wv, Wv @ _blockdiag(np.asarray(a_s)),
             Wv @ _blockdiag(np.asarray(a_d))], 1)
        row = np.zeros((2, 192), np.float32)
        row[:, 128:132] = SENT_ALS
        sent = row.view(np.uint16).view(ml_dtypes.bfloat16)  # [2, 384]
        return (Waug, np.asarray(sW, np.float32),
                np.asarray(cb) + np.asarray(sb), sent, None)

    Ls = [
        prep01(inp["conv0_W"], inp["conv0_as"], inp["conv0_ad"], inp["conv0_b"],
               inp["skip0_W"], inp["skip0_b"], inp["bn0_g"], inp["bn0_b"],
               inp["bn0_m"], inp["bn0_v"]),
        prep01(inp["conv1_W"], inp["conv1_as"], inp["conv1_ad"], inp["conv1_b"],
               inp["skip1_W"], inp["skip1_b"], inp["bn1_g"], inp["bn1_b"],
               inp["bn1_m"], inp["bn1_v"]),
        prep2(inp["conv2_W"], inp["conv2_as"], inp["conv2_ad"], inp["conv2_b"],
              inp["skip2_W"], inp["skip2_b"]),
    ]

    rep = np.zeros((len(KLIST), 16, P), np.float32)
    for ki, k in enumerate(KLIST):
        for p in range(P):
            rep[ki, p // k, p] = 1.0
    rep_np = rep.reshape(len(KLIST) * 16, P)
    svar_np = SVAR_NP.astype(ml_dtypes.bfloat16)

    h = x
    for li in range(3):
        F = IN if li == 0 else 64
        OUTW = 64 if li < 2 else 256
        mean_heads = li == 2
        Waug, skipWf, biasv, sent, _ = Ls[li]
        lkey = ("nc", li, F, OUTW, N, E)
        if lkey not in _CACHE:
            _CACHE[lkey] = build_layer(
                shared, F, OUTW, relu=not mean_heads, mean_heads=mean_heads,
                tcap=64 if not mean_heads else 36,
                gb=8 if not mean_heads else 3)
        nck = _CACHE[lkey]

        hT_full = np.zeros((F, Npad), ml_dtypes.bfloat16)
        hT_full[:, :N] = h.T.astype(ml_dtypes.bfloat16)
        base = {
            "hT": hT_full,
            "Waug": Waug.astype(ml_dtypes.bfloat16),
            "skipW": skipWf.astype(ml_dtypes.bfloat16),
            "biasR": np.tile(biasv.astype(np.float32), (P, 1)),
            "svar": svar_np,
            "rep": rep_np,
            "sent": np.asarray(sent, ml_dtypes.bfloat16),
        }
        in_maps = []
        for c in range(NC):
            vmap = plans[c]["vmap"]
            hTow = np.zeros((F, Dp), ml_dtypes.bfloat16)
            valid = vmap >= 0
            hTow[:, valid] = h[c * D + vmap[valid]].T.astype(ml_dtypes.bfloat16)
            in_maps.append(dict(base, hTow=hTow,
                                idx_lo=plans[c]["idx_lo"],
                                idx_hi=plans[c]["idx_hi"]))
        import time as _time
        _t0 = _time.time()
        res = run_bass_kernel_spmd(nck, in_maps, core_ids=list(range(NC)))
        if res.exec_time_ns:
            _EXEC_NS.append(res.exec_time_ns)
        print(f"  layer {li} run wall: {_time.time()-_t0:.1f}s", flush=True)
        hn = np.zeros((N, 64), np.float32)
        for c in range(NC):
            vmap = plans[c]["vmap"]
            valid = vmap >= 0
            yv = (res.results[c]["y"].reshape(P, Dp // P, 64)
                  .transpose(1, 0, 2).reshape(Dp, 64))
            hn[c * D + vmap[valid]] = yv[valid]
        h = hn
        _DBG.append(h)
    return h



# revision 15
# speedup vs baseline: 1.2610x; 1.2610x over previous
"""GAT (3-layer, PyG-style) Trainium2 Bass kernel, 8-core dst-sharded.

Self-contained: takes full inputs, shards internally, returns full output.

Design:
  - dst nodes sharded across 8 cores (graph parallel per the sharding hint).
  - Per layer (3 SPMD launches; host only shards/permutes/transposes/casts
    between them):
    dense phase: node-major DRAM gather-table rows [hW*bn_s (bf16) | al_s f32]
      built by PE matmuls (bf16 lhsT = h^T chunks, rhs = W_aug = [W | W@a_s |
      W@a_d]), al_s carried in-row for ALL layers (row = 128 bf16 for the
      64-wide layers, 384 bf16 for the concat=False layer), plus dense al_d.
    edge phase: padded-CSR slots (K slots per dst per src-half, K bucketed by
      max src-half degree), dma_gather of one table row per slot,
      ex = exp(leaky_relu(al_s + al_d)) written back into the row, messages
      scaled by ex, segment-sum via per-k wide selection matrices (column-
      sliced per shift) matmul-accumulating [msg_sums | sum_ex] per 128-dst
      window in PSUM; windows drained in batches (7/bank or 3 banks) to
      amortize DVE overheads; drain divides by sum_ex and adds skip matmul +
      bias (+BN fold, +ReLU on the scalar engine).
  - src space is split into lo/hi halves with separate table tensors so
    dma_gather's int16 indices stay < 32768 and edge gathers can overlap the
    tail of the dense phase; padded slots gather a sentinel row whose
    al_s = -40 (=> ex ~ 0) and whose message contribution ~ 0.
"""
import numpy as np
import ml_dtypes

import concourse.bacc as bacc
import concourse.masks as masks
import concourse.mybir as mybir
import concourse.tile as tile
from concourse.alu_op_type import AluOpType
from concourse.bass_utils import run_bass_kernel_spmd

BF16 = mybir.dt.bfloat16
F32 = mybir.dt.float32
I16 = mybir.dt.int16

NC = 8
KLIST = (8, 16, 32, 64, 128)
P = 128
EPS = 1e-5
SENT_ALS = -40.0


def _round_up(x, m):
    return (x + m - 1) // m * m


# ----------------------------------------------------------------- planning

def build_plan(src, dst, N):
    D = N // NC
    HALF = N // 2
    core = dst // D
    dloc = dst % D
    half = (src >= HALF).astype(np.int64)

    deg = np.zeros((NC, D, 2), np.int64)
    np.add.at(deg, (core, dloc, half), 1)
    mx = deg.max(axis=2)  # [NC, D]
    Kd = np.select([mx <= 8, mx <= 16, mx <= 32, mx <= 64], [8, 16, 32, 64], 128)
    assert mx.max() <= 128, f"degree bucket overflow: {mx.max()}"

    nK = {k: _round_up(int((Kd == k).sum(axis=1).max()), 16) for k in KLIST}
    Dp = sum(nK.values())
    nK[8] += (-Dp) % 128
    Dp = sum(nK.values())
    off = {}
    o = 0
    for k in KLIST:
        off[k] = o
        o += nK[k]

    slabs = []
    for k in KLIST:
        q = P // k
        for i in range(nK[k] * k // P):
            slabs.append((k, off[k] + i * q))
    nslab = len(slabs)

    nwin = Dp // P
    slot0 = np.zeros(Dp, np.int64)
    for si, (k, vd0) in enumerate(slabs):
        q = P // k
        for j in range(q):
            slot0[vd0 + j] = si * P + j * k
    TOT = _round_up(nslab, 64) * P

    shared = dict(N=N, D=D, HALF=HALF, Dp=Dp, slabs=slabs,
                  nwin=nwin, TOT=TOT)

    plans = []
    for c in range(NC):
        vid = np.full(D, -1, np.int64)
        vmap = np.full(Dp, -1, np.int64)
        used = {k: 0 for k in KLIST}
        order = np.argsort(Kd[c], kind="stable")
        for d in order:
            k = int(Kd[c, d])
            pos = off[k] + used[k]
            used[k] += 1
            vid[d] = pos
            vmap[pos] = d
        em = core == c
        es = src[em]
        evd = vid[dloc[em]]
        eh = half[em]
        key = evd * 2 + eh
        si = np.argsort(key, kind="stable")
        ks = key[si]
        starts = np.zeros(2 * Dp + 1, np.int64)
        np.cumsum(np.bincount(ks, minlength=2 * Dp), out=starts[1:])
        rank = np.arange(len(ks)) - starts[ks]
        spos = slot0[evd[si]] + rank
        essorted = es[si]
        lo = np.full(TOT, HALF, np.int64)
        hi = np.full(TOT, HALF, np.int64)
        mlo = ks % 2 == 0
        lo[spos[mlo]] = essorted[mlo]
        hi[spos[~mlo]] = essorted[~mlo] - HALF
        plans.append(dict(vmap=vmap, idx_lo=_wrap16(lo), idx_hi=_wrap16(hi)))
    return shared, plans


def _wrap16(stream):
    TOT = len(stream)
    w = stream.reshape(TOT // 16, 16).T.astype(np.int16)
    return np.tile(w, (8, 1))


def _svar_wide():
    """One wide [P, 2P] selection matrix per k; the (k, s)-shifted selection
    matrix is the column window [P - s*q, 2P - s*q) of wide_k, since
    wide_k[p, P + p//k] = 1 puts slot p's target at col s*q + p//k within
    that window."""
    mats = []
    for k in KLIST:
        m = np.zeros((P, 2 * P), np.float32)
        for p in range(P):
            m[p, P + p // k] = 1.0
        mats.append(m)
    return np.concatenate(mats, 1)  # [P, NKL*2P], partition-major contiguous


SVAR_NP = _svar_wide()
NKL = len(KLIST)


# ------------------------------------------------------------- kernel build

def build_layer(shared, F, OUTW, relu, mean_heads, tcap, gb, dbg_stage=99):
    N, HALF, Dp = shared["N"], shared["HALF"], shared["Dp"]
    nwin = shared["nwin"]
    slabs = shared["slabs"]
    TOT = shared["TOT"]
    TOT_lo = TOT_hi = TOT
    nslab = len(slabs)
    first_slab = {}
    last_slab = {}
    for i, (k, vd0) in enumerate(slabs):
        w = vd0 // P
        first_slab.setdefault(w, i)
        last_slab[w] = i
    groups = []
    s0 = 0
    while s0 < nslab:
        groups.append((s0, min(s0 + tcap, nslab)))
        s0 += tcap

    RW = 128
    NA = 72
    NAW = OUTW + 4
    MW = 260  # mean_heads msg width: 4 head-scaled x copies + 4 ex
    PADW = 128
    Npad = _round_up(N, P)
    nchunk = Npad // P
    TROWS = Npad + 2

    nc = bacc.Bacc("TRN2", target_bir_lowering=False, debug=False)
    hT = nc.dram_tensor("hT", [F, Npad], BF16, kind="ExternalInput")
    hTow = nc.dram_tensor("hTow", [F, Dp], BF16, kind="ExternalInput")
    Waug = nc.dram_tensor("Waug", [F, NA], BF16, kind="ExternalInput")
    skipW = nc.dram_tensor("skipW", [F, 64], BF16, kind="ExternalInput")
    biasR = nc.dram_tensor("biasR", [P, 64], F32, kind="ExternalInput")
    svar_in = nc.dram_tensor("svar", [P, NKL * 2 * P], BF16,
                             kind="ExternalInput")
    rep_in = nc.dram_tensor("rep", [len(KLIST) * 16, P], F32, kind="ExternalInput")
    if mean_heads:
        wq_in = nc.dram_tensor("wq", [P, 2 * 64], BF16, kind="ExternalInput")
    sent_in = nc.dram_tensor("sent", [2, RW], BF16, kind="ExternalInput")
    idx_lo = nc.dram_tensor("idx_lo", [P, TOT_lo // 16], I16,
                            kind="ExternalInput")
    idx_hi = nc.dram_tensor("idx_hi", [P, TOT_hi // 16], I16,
                            kind="ExternalInput")

    TROWS_H = Npad - HALF + 2
    table_lo = nc.dram_tensor("table_lo", [TROWS_H, RW], BF16, kind="Internal")
    table_hi = nc.dram_tensor("table_hi", [TROWS_H, RW], BF16, kind="Internal")
    aldv_d = nc.dram_tensor("aldv", [Dp, 4], F32, kind="Internal")
    y_out = nc.dram_tensor("y", [P, nwin * 64], F32, kind="ExternalOutput")

    def table_row_ranges(n0, n1):
        """split [n0,n1) at HALF into (tensor, node range, dram row) pieces."""
        out = []
        cuts = sorted({n0, min(max(HALF, n0), n1), n1})
        for a, b in zip(cuts, cuts[1:]):
            if a >= b:
                continue
            if a < HALF:
                out.append((table_lo, a, b, a))
            else:
                out.append((table_hi, a, b, a - HALF))
        return out

    with tile.TileContext(nc) as tc:
        with (
            tc.tile_pool(name="const", bufs=1) as cp,
            tc.tile_pool(name="ybuf", bufs=1) as yp,
        ):
            waug_sb = cp.tile([F, NA], BF16)
            nc.sync.dma_start(waug_sb[:], Waug[:])
            skipw_sb = cp.tile([F, 64], BF16)
            nc.sync.dma_start(skipw_sb[:], skipW[:])
            bias_sb = cp.tile([P, 1, 64], F32)
            nc.sync.dma_start(bias_sb[:],
                              biasR[:].rearrange("p (x c) -> p x c", x=1))
            svar_sb = cp.tile([P, NKL * 2 * P], BF16)
            nc.sync.dma_start(svar_sb[:], svar_in[:])
            rep_sb = cp.tile([16, len(KLIST), P], F32)
            nc.sync.dma_start(rep_sb[:],
                              rep_in[:].rearrange("(v p) c -> p v c", p=16))
            hTow_sb = cp.tile([F, Dp], BF16)
            nc.scalar.dma_start(hTow_sb[:], hTow[:])
            if mean_heads:
                wq_sb = cp.tile([P, 2, 64], BF16)
                nc.sync.dma_start(wq_sb[:],
                                  wq_in[:].rearrange("p (j c) -> p j c", j=2))
                ident_sb = cp.tile([P, P], BF16)
                masks.make_identity(nc, ident_sb[:])
            y_sb = yp.tile([P, nwin, 64], F32)

            # ---------------- dense phase: gather table + dense al_d
            with (
                tc.tile_pool(name="dstage", bufs=3) as dsp,
                tc.tile_pool(name="pdense", bufs=2, space="PSUM") as pd,
                tc.tile_pool(name="pal", bufs=1, space="PSUM") as pal,
            ):
                sent_sb = dsp.tile([2, RW], BF16, tag="sent")
                nc.scalar.dma_start(sent_sb[:], sent_in[:])
                nc.scalar.dma_start(table_lo[HALF: HALF + 1, :],
                                    sent_sb[0:1, :])
                nc.scalar.dma_start(table_hi[HALF: HALF + 1, :],
                                    sent_sb[1:2, :])

                ndc = Dp // P
                alps = pal.tile([P, ndc * 4], F32, space="PSUM")
                for i in range(ndc):
                    nc.tensor.matmul(
                        alps[:, i * 4: (i + 1) * 4],
                        hTow_sb[:, i * P: (i + 1) * P],
                        waug_sb[:, NA - 4: NA],
                        start=True, stop=True,
                    )
                alsb = dsp.tile([P, ndc * 4], F32, tag="alsb")
                nc.vector.tensor_copy(alsb[:], alps[:])
                nc.scalar.dma_start(
                    aldv_d[:].rearrange("(i p) h -> p i h", p=P),
                    alsb[:].rearrange("p (i h) -> p i h", h=4),
                )

                UW = OUTW + 8  # used row prefix: msg bf16 + 4 f32 al_s
                sgb = 4 * gb
                for sg0 in range(0, nchunk, sgb):
                    sg1 = min(sg0 + sgb, nchunk)
                    stage = dsp.tile([F, sgb * P], BF16, tag="stage")
                    nc.sync.dma_start(stage[:, : (sg1 - sg0) * P],
                                      hT[:, sg0 * P: sg1 * P])
                    tstage = dsp.tile([P, sgb, RW], BF16, tag="tstage")
                    tf32 = tstage[:].bitcast(F32)
                    for g0 in range(sg0, sg1, gb):
                        g1 = min(g0 + gb, sg1)
                        ng = g1 - g0
                        c0 = g0 - sg0
                        dps = pd.tile([P, gb * PADW], F32, space="PSUM",
                                      tag="dps")
                        for i in range(ng):
                            nc.tensor.matmul(
                                dps[:, i * PADW: i * PADW + NA],
                                stage[:, (c0 + i) * P: (c0 + i + 1) * P],
                                waug_sb[:],
                                start=True, stop=True,
                            )
                        dv = dps[:].rearrange("p (i w) -> p i w", w=PADW)
                        nc.vector.tensor_copy(
                            tstage[:, c0: c0 + ng, 0:OUTW],
                            dv[:, :ng, 0:OUTW])
                        nc.vector.tensor_copy(
                            tf32[:, c0: c0 + ng, OUTW // 2: OUTW // 2 + 4],
                            dv[:, :ng, OUTW: OUTW + 4])
                    for (tbl, a, b, r) in table_row_ranges(
                            sg0 * P, min(sg1 * P, N)):
                        # table_hi goes out on the Activation queue so the
                        # lo gathers' DMA-sem wait only covers table_lo
                        eng = nc.scalar if tbl is table_hi else nc.sync
                        # emit aligned middle as one DMA; partial chunks solo
                        n0 = a
                        while n0 < b:
                            if n0 % P != 0 or b - n0 < P:
                                n1 = min(b, n0 - n0 % P + P)
                                ci = n0 // P - sg0
                                eng.dma_start(
                                    tbl[r + n0 - a: r + n1 - a, 0:UW],
                                    tstage[n0 % P: n0 % P + (n1 - n0), ci,
                                           0:UW],
                                )
                            else:
                                n1 = n0 + (b - n0) // P * P
                                ci = n0 // P - sg0
                                m = (n1 - n0) // P
                                eng.dma_start(
                                    tbl[r + n0 - a: r + n1 - a, 0:UW]
                                    .rearrange("(i p) w -> p i w", p=P),
                                    tstage[:, ci: ci + m, 0:UW],
                                )
                            n0 = n1

            # ---------------- edge phase
            NB = 2 if mean_heads else 7          # windows per drain batch
            BSTRIDE = 512 if mean_heads else 68  # f32 cols per window slot
            with (
                tc.tile_pool(name="gpool",
                             bufs=2 if mean_heads else 3) as gp,
                tc.tile_pool(name="mpool", bufs=2) as mp,
                tc.tile_pool(name="spool", bufs=2) as ssp,
                tc.tile_pool(name="pwin", bufs=2, space="PSUM") as pw,
                tc.tile_pool(name="palde", bufs=1 if mean_heads else 2,
                             space="PSUM") as pa,
                tc.tile_pool(name="psk", bufs=1 if mean_heads else 2,
                             space="PSUM") as pk,
                tc.tile_pool(name="ptp", bufs=2, space="PSUM") as ptp,
            ):
                win_ps = {}
                for (s0, s1) in groups:
                    T = s1 - s0
                    g_lo = gp.tile([P, tcap, RW], BF16, tag="Glo")
                    g_hi = gp.tile([P, tcap, RW], BF16, tag="Ghi")
                    il_t = ssp.tile([P, tcap * 8], I16, tag="il")
                    ih_t = ssp.tile([P, tcap * 8], I16, tag="ih")
                    nc.scalar.dma_start(il_t[:, : T * 8],
                                        idx_lo[:, s0 * 8:(s0 + T) * 8])
                    nc.scalar.dma_start(ih_t[:, : T * 8],
                                        idx_hi[:, s0 * 8:(s0 + T) * 8])
                    nc.gpsimd.dma_gather(
                        g_lo[:, :T], table_lo[0: HALF + 1, :],
                        il_t[:, : T * 8], T * P, T * P, RW,
                        single_packet=False)
                    nc.gpsimd.dma_gather(
                        g_hi[:, :T], table_hi[0: HALF + 1, :],
                        ih_t[:, : T * 8], T * P, T * P, RW,
                        single_packet=False)

                    alde = ssp.tile([P, tcap, 4], F32, tag="alde")
                    i = s0
                    while i < s1:
                        k = slabs[i][0]
                        j = i
                        while j < s1 and slabs[j][0] == k:
                            j += 1
                        q = P // k
                        run = j - i
                        vb = slabs[i][1]
                        cont = ssp.tile([16, tcap, 4], F32, tag="cont")
                        nc.scalar.dma_start(
                            cont[:q, :run, :],
                            aldv_d[vb: vb + run * q, :].rearrange(
                                "(t j) h -> j t h", j=q),
                        )
                        aps = pa.tile([P, tcap * 4], F32, space="PSUM",
                                      tag="aldeps")
                        nc.tensor.matmul(
                            aps[:, : run * 4],
                            rep_sb[:q, KLIST.index(k), :],
                            cont[:q, :run, :].rearrange("j t h -> j (t h)"),
                            start=True, stop=True,
                        )
                        nc.vector.tensor_copy(
                            alde[:, i - s0: j - s0, :],
                            aps[:, : run * 4].rearrange("p (t h) -> p t h",
                                                        h=4),
                        )
                        i = j

                    z_t = ssp.tile([P, 2 * tcap, 4], F32, tag="z")
                    msgs = {}
                    for h in range(2):
                        gs = (g_lo if h == 0 else g_hi)[:, :T, :]
                        gf = (g_lo if h == 0 else g_hi)[:].bitcast(F32)
                        zs = z_t[:, h * tcap: h * tcap + T, :]
                        nc.vector.tensor_tensor(
                            zs, gf[:, :T, OUTW // 2: OUTW // 2 + 4],
                            alde[:, :T, :], AluOpType.add)
                        nc.vector.scalar_tensor_tensor(
                            zs, zs, 0.2, zs, AluOpType.mult, AluOpType.max)
                        if mean_heads:
                            # per-head-scaled copies of x into a msg tile:
                            # [x*ex0 | x*ex1 | x*ex2 | x*ex3 | ex0..ex3]
                            m_t = mp.tile([P, tcap, MW], BF16,
                                          tag="Mlo" if h == 0 else "Mhi")
                            msgs[h] = m_t
                            nc.scalar.activation(
                                m_t[:, :T, 256: 260], zs,
                                mybir.ActivationFunctionType.Exp)
                            for hh in range(4):
                                eng = nc.vector if hh < 2 else nc.gpsimd
                                eng.tensor_tensor(
                                    m_t[:, :T, hh * 64: (hh + 1) * 64],
                                    gs[:, :, 0:64],
                                    m_t[:, :T, 256 + hh: 257 + hh]
                                    .to_broadcast([P, T, 64]),
                                    AluOpType.mult,
                                )
                        else:
                            nc.scalar.activation(
                                gs[:, :, OUTW: OUTW + 4], zs,
                                mybir.ActivationFunctionType.Exp)
                            for hh in range(4):
                                ex_ap = gs[:, :, OUTW + hh: OUTW + hh + 1]
                                nc.vector.tensor_tensor(
                                    gs[:, :, hh * (OUTW // 4):
                                       (hh + 1) * (OUTW // 4)],
                                    gs[:, :, hh * (OUTW // 4):
                                       (hh + 1) * (OUTW // 4)],
                                    ex_ap.to_broadcast([P, T, OUTW // 4]),
                                    AluOpType.mult,
                                )

                    for i in range(s0, s1):
                        k, vd0 = slabs[i]
                        w = vd0 // P
                        b = w // NB
                        if b not in win_ps:
                            win_ps[b] = pw.tile([P, NB * BSTRIDE], F32,
                                                space="PSUM", tag="win",
                                                name=f"winb{b}")
                        wb = (w % NB) * BSTRIDE
                        off = P - (vd0 % P)
                        ki = KLIST.index(k)
                        sv = svar_sb[:, ki * 2 * P + off: ki * 2 * P + off + P]
                        for h in range(2):
                            st = (h == 0) and (first_slab[w] == i)
                            fin = (h == 1) and (last_slab[w] == i)
                            t = i - s0
                            if mean_heads:
                                nc.tensor.matmul(
                                    win_ps[b][:, wb: wb + MW], sv,
                                    msgs[h][:, t, 0:MW],
                                    start=st, stop=fin, skip_group_check=True)
                            else:
                                gh = g_lo if h == 0 else g_hi
                                nc.tensor.matmul(
                                    win_ps[b][:, wb: wb + NAW], sv,
                                    gh[:, t, 0:NAW],
                                    start=st, stop=fin, skip_group_check=True)
                        w1 = min((b + 1) * NB, nwin) - 1
                        if w != w1 or last_slab[w] != i or dbg_stage < 6:
                            continue
                        pwb = win_ps.pop(b)
                        w0 = b * NB
                        nb = w1 - w0 + 1
                        pv = pwb[:, : nb * BSTRIDE].rearrange(
                            "p (b c) -> p b c", c=BSTRIDE)
                        sk = pk.tile([P, NB * 64], F32, space="PSUM",
                                     tag="skps")
                        rec = ssp.tile([P, NB, 4], F32, tag="rec")
                        yw = y_sb[:, w0: w0 + nb, :]
                        if mean_heads:
                            # normalize per head into bf16, transpose to
                            # feature-major, apply stacked W/4 + skip on PE
                            nc.vector.reciprocal(rec[:, :nb, :],
                                                 pv[:, :, 256:260])
                            nrm = ssp.tile([P, NB, 256], BF16, tag="nrm")
                            for hh in range(4):
                                nc.vector.tensor_tensor(
                                    nrm[:, :nb, hh * 64: (hh + 1) * 64],
                                    pv[:, :, hh * 64: (hh + 1) * 64],
                                    rec[:, :nb, hh: hh + 1].to_broadcast(
                                        [P, nb, 64]),
                                    AluOpType.mult)
                            for j in range(nb):
                                tp = ptp.tile([P, 2, P], BF16, space="PSUM",
                                              tag="tp")
                                for c in range(2):
                                    nc.tensor.transpose(
                                        tp[:, c, :],
                                        nrm[:, j, c * P: (c + 1) * P],
                                        ident_sb[:])
                                aggT = ssp.tile([P, 2, P], BF16, tag="aggT")
                                nc.vector.tensor_copy(aggT[:], tp[:])
                                sks = sk[:, j * 64: (j + 1) * 64]
                                nc.tensor.matmul(sks, aggT[:, 0, :],
                                                 wq_sb[:, 0, :],
                                                 start=True, stop=False)
                                nc.tensor.matmul(sks, aggT[:, 1, :],
                                                 wq_sb[:, 1, :],
                                                 start=False, stop=False)
                                nc.tensor.matmul(
                                    sks,
                                    hTow_sb[:, (w0 + j) * P: (w0 + j + 1) * P],
                                    skipw_sb[:], start=False, stop=True)
                            skv = sk[:, : nb * 64].rearrange(
                                "p (b c) -> p b c", c=64)
                            nc.vector.tensor_tensor(
                                yw, skv, bias_sb[:].to_broadcast([P, nb, 64]),
                                AluOpType.add)
                        else:
                            for j in range(nb):
                                nc.tensor.matmul(
                                    sk[:, j * 64: (j + 1) * 64],
                                    hTow_sb[:, (w0 + j) * P: (w0 + j + 1) * P],
                                    skipw_sb[:], start=True, stop=True)
                            skv = sk[:, : nb * 64].rearrange(
                                "p (b c) -> p b c", c=64)
                            nc.vector.reciprocal(rec[:, :nb, :],
                                                 pv[:, :, OUTW: OUTW + 4])
                            for hh in range(4):
                                nc.vector.tensor_tensor(
                                    yw[:, :, hh * 16: (hh + 1) * 16],
                                    pv[:, :, hh * 16: (hh + 1) * 16],
                                    rec[:, :nb, hh: hh + 1].to_broadcast(
                                        [P, nb, 16]),
                                    AluOpType.mult)
                            nc.vector.tensor_tensor(yw, yw, skv, AluOpType.add)
                            nc.vector.tensor_tensor(
                                yw, yw, bias_sb[:].to_broadcast([P, nb, 64]),
                                AluOpType.add)
                            if relu:
                                nc.scalar.activation(
                                    yw, yw,
                                    mybir.ActivationFunctionType.Relu)
                        # write this batch's y slice now so the final DMA
                        # isn't a serial tail after the last drain
                        nc.sync.dma_start(
                            y_out[:, w0 * 64: (w0 + nb) * 64].rearrange(
                                "p (w c) -> p w c", c=64),
                            y_sb[:, w0: w0 + nb, :])
    nc.compile()
    return nc


# ------------------------------------------------------------------ driver

_CACHE = {}
_DBG = []
_EXEC_NS = []


def _blockdiag(a):
    H, C = a.shape
    m = np.zeros((H * C, H), np.float32)
    for hh in range(H):
        m[hh * C: (hh + 1) * C, hh] = a[hh]
    return m


def _sent01():
    row = np.zeros((2, 64), np.float32)
    row[:, 32:36] = SENT_ALS
    return row.view(np.uint16).view(ml_dtypes.bfloat16)  # [2, 128]


def kernel(**inp):
    x = np.asarray(inp["x"], np.float32)
    ei = np.asarray(inp["edge_index"], np.int64)
    N, IN = x.shape
    E = ei.shape[1]

    loops = np.arange(N, dtype=np.int64)
    src = np.concatenate([ei[0], loops])
    dst = np.concatenate([ei[1], loops])

    pkey = ("plan", N, E, hash(ei.tobytes()))
    if pkey not in _CACHE:
        _CACHE[pkey] = build_plan(src, dst, N)
    shared, plans = _CACHE[pkey]
    Dp, D = shared["Dp"], shared["D"]
    Npad = _round_up(N, P)

    def prep01(Wv, a_s, a_d, cb, sW, sb, g, b, m, v):
        Wv, sW = np.asarray(Wv, np.float32), np.asarray(sW, np.float32)
        bns = (np.asarray(g) / np.sqrt(np.asarray(v) + EPS)).astype(np.float32)
        bnt = (np.asarray(b) - np.asarray(m) * bns).astype(np.float32)
        Waug = np.concatenate(
            [Wv * bns[None, :], Wv @ _blockdiag(np.asarray(a_s)),
             Wv @ _blockdiag(np.asarray(a_d))], 1)
        return (Waug, sW * bns[None, :], np.asarray(cb) * bns
                + np.asarray(sb) * bns + bnt, _sent01(), None)

    def prep2(Wv, a_s, a_d, cb, sW, sb):
        # layer 2 aggregates in INPUT space: rows carry [x | al_s]; the
        # stacked W/4 is applied post-normalization in the drain.
        Wv = np.asarray(Wv, np.float32)  # [64, 256]
        Waug = np.concatenate(
            [np.eye(64, dtype=np.float32),
             Wv @ _blockdiag(np.asarray(a_s)),
             Wv @ _blockdiag(np.asarray(a_d))], 1)  # [64, 72]
        # Wq[p, j*64+c] = Wstack[j*128+p, c], Wstack[h*64+u, c] = Wv[u, h*64+c]/4
        Wstack = np.zeros((256, 64), np.float32)
        for h in range(4):
            Wstack[h * 64:(h + 1) * 64, :] = Wv[:, h * 64:(h + 1) * 64] / 4.0
        Wq = np.concatenate([Wstack[0:128, :], Wstack[128:256, :]], 1)
        return (Waug, np.asarray(sW, np.float32),
                np.asarray(cb) + np.asarray(sb), _sent01(), Wq)

    Ls = [
        prep01(inp["conv0_W"], inp["conv0_as"], inp["conv0_ad"], inp["conv0_b"],
               inp["skip0_W"], inp["skip0_b"], inp["bn0_g"], inp["bn0_b"],
               inp["bn0_m"], inp["bn0_v"]),
        prep01(inp["conv1_W"], inp["conv1_as"], inp["conv1_ad"], inp["conv1_b"],
               inp["skip1_W"], inp["skip1_b"], inp["bn1_g"], inp["bn1_b"],
               inp["bn1_m"], inp["bn1_v"]),
        prep2(inp["conv2_W"], inp["conv2_as"], inp["conv2_ad"], inp["conv2_b"],
              inp["skip2_W"], inp["skip2_b"]),
    ]

    rep = np.zeros((len(KLIST), 16, P), np.float32)
    for ki, k in enumerate(KLIST):
        for p in range(P):
            rep[ki, p // k, p] = 1.0
    rep_np = rep.reshape(len(KLIST) * 16, P)
    svar_np = SVAR_NP.astype(ml_dtypes.bfloat16)

    h = x
    for li in range(3):
        F = IN if li == 0 else 64
        OUTW = 64
        mean_heads = li == 2
        Waug, skipWf, biasv, sent, wq = Ls[li]
        lkey = ("nc", li, F, OUTW, N, E)
        if lkey not in _CACHE:
            _CACHE[lkey] = build_layer(
                shared, F, OUTW, relu=not mean_heads, mean_heads=mean_heads,
                tcap=64 if not mean_heads else 48, gb=8)
        nck = _CACHE[lkey]

        hT_full = np.zeros((F, Npad), ml_dtypes.bfloat16)
        hT_full[:, :N] = h.T.astype(ml_dtypes.bfloat16)
        base = {
            "hT": hT_full,
            "Waug": Waug.astype(ml_dtypes.bfloat16),
            "skipW": skipWf.astype(ml_dtypes.bfloat16),
            "biasR": np.tile(biasv.astype(np.float32), (P, 1)),
            "svar": svar_np,
            "rep": rep_np,
            "sent": np.asarray(sent, ml_dtypes.bfloat16),
        }
        if mean_heads:
            base["wq"] = wq.astype(ml_dtypes.bfloat16)
        in_maps = []
        for c in range(NC):
            vmap = plans[c]["vmap"]
            hTow = np.zeros((F, Dp), ml_dtypes.bfloat16)
            valid = vmap >= 0
            hTow[:, valid] = h[c * D + vmap[valid]].T.astype(ml_dtypes.bfloat16)
            in_maps.append(dict(base, hTow=hTow,
                                idx_lo=plans[c]["idx_lo"],
                                idx_hi=plans[c]["idx_hi"]))
        import time as _time
        _t0 = _time.time()
        res = run_bass_kernel_spmd(nck, in_maps, core_ids=list(range(NC)))
        if res.exec_time_ns:
            _EXEC_NS.append(res.exec_time_ns)
        print(f"  layer {li} run wall: {_time.time()-_t0:.1f}s", flush=True)
        hn = np.zeros((N, 64), np.float32)
        for c in range(NC):
            vmap = plans[c]["vmap"]
            valid = vmap >= 0
            yv = (res.results[c]["y"].reshape(P, Dp // P, 64)
                  .transpose(1, 0, 2).reshape(Dp, 64))
            hn[c * D + vmap[valid]] = yv[valid]
        h = hn
        _DBG.append(h)
    return h

